# revision 1
# baseline (speedup 1.0000x reference)
"""Trainium2 Bass kernel for nn_BaselineOut (article/option additive-attention MRC head).

Contract: kernel(**inputs) takes FULL unsharded inputs (numpy), returns FULL
[32, 5] float32 logits.  Internally: data-parallel over batch across 8 cores
(4 batch items per core), all params replicated.

Math notes (vs reference):
  - oqc gather is done as a one-hot matmul on device (host only encodes the
    int indices as a one-hot fp32 matrix - a layout/encoding transform).
  - V-projection is pulled out of the attention sum by linearity:
        sum_l softmax_l * (V @ Vw^T + Vb) = (sum_l softmax_l * V) @ Vw^T + Vb
    so the [B*L,H]x[H,H] V matmul collapses to a weighted sum over L plus a
    tiny [B,H]x[H,H] matmul.
  - Consecutive linear maps with no nonlinearity between are constant-folded
    on host (weight-weight products):
      * aq -> Qp_d: one matmul with Wqv = d_Qw @ a_Vw^T and a folded bias.
      * feats -> logits: per-option folded weights Ff_o = d_Vw^T @ f_w[:,o]^T.
  - softmax logit bias (vb) is dropped: softmax is shift-invariant.
  - exp is computed without max-subtraction: |logit| <= ||vw||_1 ~ 36, well
    inside fp32 exp range.
  - Large matmuls run with float32r operands (full-rate fp32 on the PE);
    f32r must never be a DRAM I/O dtype (crashes NRT) - the f32->f32r cast
    happens in SWDGE DMAs.
"""

import functools
import sys

import numpy as np

sys.path.insert(0, "/opt/trn_rl_repo")

import concourse.bass as bass  # noqa: E402
from concourse import bacc  # noqa: E402
import concourse.tile as tile  # noqa: E402
from concourse import mybir  # noqa: E402
from concourse.bass import ds, ts  # noqa: E402

B, LA, LQ, LO, H, OUT = 32, 2048, 64, 32, 1024, 5
NCORES = 8
BL = B // NCORES  # 4 batch items per core
NOPT = 5
F32 = mybir.dt.float32
F32R = mybir.dt.float32r
LT = 512  # article l-tile (free dim of the big matmuls)
NLT = LA // LT  # 4
C = H // 128  # 8 h-chunks
BO = BL * NOPT  # 20 (b, option) pairs per core
AF = mybir.ActivationFunctionType
ALU = mybir.AluOpType
AX = mybir.AxisListType
OUTP = 8  # final-linear out dim padded even for f32r


def build_nc() -> bass.Bass:
    nc = bacc.Bacc("TRN2", target_bir_lowering=False, debug=False)

    # ---- DRAM I/O (per-core shard; names are the in_map keys) ----
    artT = nc.dram_tensor("artT", [BL, H, LA], F32, kind="ExternalInput").ap()
    optT = nc.dram_tensor("optT", [BL, H, NOPT, LO], F32, kind="ExternalInput").ap()
    qcd = nc.dram_tensor("qc", [BL, LQ, H], F32, kind="ExternalInput").ap()
    ohd = nc.dram_tensor("oh", [LQ, BL], F32, kind="ExternalInput").ap()
    wQa = nc.dram_tensor("aQwT", [H, H], F32, kind="ExternalInput").ap()
    wKa = nc.dram_tensor("aKwT", [H, H], F32, kind="ExternalInput").ap()
    wQV = nc.dram_tensor("qvwT", [H, H], F32, kind="ExternalInput").ap()
    wKd = nc.dram_tensor("dKwT", [H, H], F32, kind="ExternalInput").ap()
    vwad = nc.dram_tensor("vwaT", [128, C], F32, kind="ExternalInput").ap()
    vwdd = nc.dram_tensor("vwdT", [128, C], F32, kind="ExternalInput").ap()
    qkbd = nc.dram_tensor("qkbT", [128, C], F32, kind="ExternalInput").ap()
    qvbd = nc.dram_tensor("qvbT", [128, C], F32, kind="ExternalInput").ap()
    fwd = nc.dram_tensor("fwT", [128, NOPT, C, OUTP], F32, kind="ExternalInput").ap()
    fbd = nc.dram_tensor("fb", [BL, OUTP], F32, kind="ExternalInput").ap()
    onesd = nc.dram_tensor("ones1", [1, 128], F32, kind="ExternalInput").ap()
    outd = nc.dram_tensor("out", [BL, OUT], F32, kind="ExternalOutput").ap()

    with (
        tile.TileContext(nc) as tc,
        nc.allow_low_precision(reason="float32r is 4-byte; PE accumulates fp32"),
    ):
        with (
            tc.tile_pool(name="stream", bufs=3) as stream,
            tc.tile_pool(name="wbig", bufs=3) as wbig,
            tc.tile_pool(name="mpool", bufs=3) as mpool,
            tc.tile_pool(name="spool", bufs=2) as spool,
            tc.tile_pool(name="rpool", bufs=2) as rpool,
            tc.tile_pool(name="rdpool", bufs=1) as rdpool,
            tc.tile_pool(name="ubuf", bufs=2) as ubuf,
            tc.tile_pool(name="scratch", bufs=1) as scratch,
            tc.tile_pool(name="one", bufs=1) as one,
            tc.tile_pool(name="pacc", bufs=4, space="PSUM") as pacc,
            tc.tile_pool(name="prow", bufs=2, space="PSUM") as prow,
            tc.tile_pool(name="psml", bufs=2, space="PSUM") as psml,
        ):
            # ---------- small constant loads (ahead of big weights) ----------
            vwa = one.tile([128, C], F32R, tag="vwa")
            nc.gpsimd.dma_start(out=vwa, in_=vwad)
            vwd = one.tile([128, C], F32R, tag="vwd")
            nc.gpsimd.dma_start(out=vwd, in_=vwdd)
            ones = one.tile([1, 128], F32R, tag="ones")
            nc.gpsimd.dma_start(out=ones, in_=onesd)
            qkb = one.tile([128, C], F32, tag="qkb")
            nc.sync.dma_start(out=qkb, in_=qkbd)
            qvb = one.tile([128, C], F32, tag="qvb")
            nc.sync.dma_start(out=qvb, in_=qvbd)
            fw = one.tile([128, NOPT, C, OUTP], F32R, tag="fw")
            nc.gpsimd.dma_start(out=fw, in_=fwd)
            fb = one.tile([BL, OUTP], F32, tag="fb")
            nc.sync.dma_start(out=fb, in_=fbd)
            oht = one.tile([LQ, BL], F32, tag="oht")
            nc.sync.dma_start(out=oht, in_=ohd)
            qct = stream.tile([LQ, BL, H], F32, tag="stream")
            for b in range(BL):
                nc.sync.dma_start(out=qct[:, b, :], in_=qcd[b])

            # ---------- big weights ----------
            # wk casts f32->f32r so it must use SWDGE; it heads the SWDGE queue
            # so the first Kp matmuls start as early as possible.  Plain-f32
            # weights ride HWDGE behind the small loads.
            wk = wbig.tile([128, C, H], F32R, tag="w")
            nc.gpsimd.dma_start(
                out=wk[:, :, ts(0, 128)],
                in_=wKa[:, ts(0, 128)].rearrange("(c p) o -> p c o", p=128),
            )
            wq = wbig.tile([128, C, H], F32, tag="w")
            nc.sync.dma_start(out=wq, in_=wQa.rearrange("(c p) o -> p c o", p=128))
            wqv = wbig.tile([128, C, H], F32R, tag="w")

            # ---------- gather oqc via one-hot matmul ----------
            oqcT = one.tile([128, C, BL], F32, tag="oqcT")
            for c in range(C):
                po = psml.tile([128, BL], F32, tag="sml")
                for b in range(BL):
                    nc.tensor.matmul(
                        po[:, b : b + 1],
                        lhsT=qct[:, b, ts(c, 128)],
                        rhs=oht[:, b : b + 1],
                        start=True,
                        stop=True,
                    )
                nc.vector.tensor_copy(oqcT[:, c, :], po)

            # ---------- Qp^T = aQw @ oqc^T ; article tanh bias ----------
            biasA = one.tile([128, C, BL], F32, tag="biasA")
            for co in range(C):
                pq = psml.tile([128, BL], F32, tag="sml")
                for ci in range(C):
                    nc.tensor.matmul(
                        pq,
                        lhsT=wq[:, ci, ts(co, 128)],
                        rhs=oqcT[:, ci, :],
                        start=(ci == 0),
                        stop=(ci == C - 1),
                    )
                nc.vector.tensor_scalar_add(biasA[:, co, :], pq, qkb[:, co : co + 1])

            # ---------- article branch ----------
            s_sums = one.tile([1, BL, NLT], F32, tag="s_sums")
            uTun = one.tile([128, C, BL], F32, tag="uTun")
            wdk = wbig.tile([128, C, H], F32R, tag="w")
            for b in range(BL):
                upart = ubuf.tile([128, C, NLT], F32, tag="upart")
                for lt in range(NLT):
                    T = stream.tile([128, C, LT], F32R, tag="stream")
                    nc.gpsimd.dma_start(
                        out=T,
                        in_=artT[b, :, ds(lt * LT, LT)].rearrange(
                            "(c p) l -> p c l", p=128
                        ),
                    )
                    if b == 0 and lt == 0:
                        for cw in range(1, C):
                            nc.gpsimd.dma_start(
                                out=wk[:, :, ts(cw, 128)],
                                in_=wKa[:, ts(cw, 128)].rearrange(
                                    "(c p) o -> p c o", p=128
                                ),
                            )
                    lg = prow.tile([1, LT], F32, tag="lg")
                    for co in range(C):
                        kp = pacc.tile([128, LT], F32, tag="acc")
                        for ci in range(C):
                            nc.tensor.matmul(
                                kp,
                                lhsT=wk[:, ci, ts(co, 128)],
                                rhs=T[:, ci, :],
                                start=(ci == 0),
                                stop=(ci == C - 1),
                            )
                        mt = mpool.tile([128, LT], F32R, tag="mt")
                        nc.scalar.activation(
                            mt, kp, AF.Tanh, bias=biasA[:, co, b : b + 1]
                        )
                        nc.tensor.matmul(
                            lg,
                            lhsT=vwa[:, co : co + 1],
                            rhs=mt,
                            start=(co == 0),
                            stop=(co == C - 1),
                        )
                    st = spool.tile([1, LT], F32R, tag="st")
                    nc.scalar.activation(
                        st, lg, AF.Exp, accum_out=s_sums[:, b, lt : lt + 1]
                    )
                    # replicate s~ across partitions: ones^T (x) st via PE
                    prep = pacc.tile([128, LT], F32, tag="acc")
                    nc.tensor.matmul(prep, lhsT=ones, rhs=st, start=True, stop=True)
                    srep = rpool.tile([128, LT], F32, tag="srep")
                    nc.scalar.copy(srep, prep)
                    CH = C // 2
                    for hh in range(2):
                        scr = scratch.tile([128, CH, LT], F32, tag="scr")
                        nc.vector.tensor_mul(
                            scr,
                            T[:, ds(hh * CH, CH)].bitcast(F32),
                            srep.unsqueeze(1).broadcast_to((128, CH, LT)),
                        )
                        nc.vector.tensor_reduce(
                            upart[:, ds(hh * CH, CH), lt : lt + 1],
                            scr,
                            axis=AX.X,
                            op=ALU.add,
                        )
                if b == 0:
                    # emit the wqv/dKw loads after b0's article tiles are
                    # queued: they ride SWDGE behind them and land mid-article,
                    # well before the options phase needs them.
                    nc.gpsimd.dma_start(
                        out=wqv, in_=wQV.rearrange("(c p) o -> p c o", p=128)
                    )
                    nc.gpsimd.dma_start(
                        out=wdk, in_=wKd.rearrange("(c p) o -> p c o", p=128)
                    )
                # sum the NLT partial weighted sums -> unnormalized u^T
                nc.vector.tensor_reduce(
                    uTun[:, :, b : b + 1], upart, axis=AX.X, op=ALU.add
                )

            # normalization factors: 1/sum(exp) per b, replicated to 128 parts
            ssb = one.tile([1, BL], F32, tag="ssb")
            nc.vector.tensor_reduce(ssb, s_sums, axis=AX.X, op=ALU.add)
            psb = psml.tile([128, BL], F32, tag="sml")
            nc.tensor.matmul(
                psb, lhsT=ones.bitcast(F32), rhs=ssb, start=True, stop=True
            )
            rs_rep = one.tile([128, BL], F32, tag="rs_rep")
            nc.vector.reciprocal(rs_rep, psb)

            uT = one.tile([128, C, BL], F32R, tag="uT")
            for b in range(BL):
                nc.vector.tensor_scalar_mul(
                    uT[:, :, b], uTun[:, :, b], rs_rep[:, b : b + 1]
                )

            # ---------- option tanh bias via folded Wqv = d_Qw a_Vw^T ----------
            biasO = one.tile([128, C, BL], F32, tag="biasO")
            for co in range(C):
                pq2 = psml.tile([128, BL], F32, tag="sml")
                for ci in range(C):
                    nc.tensor.matmul(
                        pq2,
                        lhsT=wqv[:, ci, ts(co, 128)],
                        rhs=uT[:, ci, :],
                        start=(ci == 0),
                        stop=(ci == C - 1),
                    )
                nc.vector.tensor_scalar_add(biasO[:, co, :], pq2, qvb[:, co : co + 1])

            # ---------- options branch ----------
            OT = stream.tile([128, C, BL, NOPT, LO], F32R, tag="stream")
            for b in range(BL):
                nc.gpsimd.dma_start(
                    out=OT[:, :, b],
                    in_=optT[b].rearrange("(c p) o l -> p c o l", p=128),
                )
            mdt = stream.tile([128, C, BL, NOPT, LO], F32R, tag="stream")
            HALF = 2 * NOPT * LO  # 320 columns (2 batch items)
            for co in range(C):
                for h in range(2):
                    kpd = pacc.tile([128, HALF], F32, tag="acc")
                    for ci in range(C):
                        nc.tensor.matmul(
                            kpd,
                            lhsT=wdk[:, ci, ts(co, 128)],
                            rhs=OT[:, ci, ds(2 * h, 2)],
                            start=(ci == 0),
                            stop=(ci == C - 1),
                        )
                    for bq in range(2):
                        b = 2 * h + bq
                        nc.scalar.activation(
                            mdt[:, co, b],
                            kpd[:, ds(bq * NOPT * LO, NOPT * LO)],
                            AF.Tanh,
                            bias=biasO[:, co, b : b + 1],
                        )

            s_d = one.tile([1, BO * LO], F32R, tag="s_d")
            for h in range(2):
                lgd = prow.tile([1, HALF], F32, tag="lg")
                for co in range(C):
                    nc.tensor.matmul(
                        lgd,
                        lhsT=vwd[:, co : co + 1],
                        rhs=mdt[:, co, ds(2 * h, 2)],
                        start=(co == 0),
                        stop=(co == C - 1),
                    )
                nc.scalar.activation(s_d[:, ds(h * HALF, HALF)], lgd, AF.Exp)

            sums_d = one.tile([1, BO], F32, tag="sums_d")
            nc.vector.tensor_reduce(
                sums_d,
                s_d.bitcast(F32).rearrange("p (bo l) -> p bo l", l=LO),
                axis=AX.X,
                op=ALU.add,
            )
            rec_d = one.tile([1, BO], F32, tag="rec_d")
            nc.vector.reciprocal(rec_d, sums_d)
            # replicate raw exp scores and 1/sum across partitions
            sdrep = rdpool.tile([128, BO * LO], F32, tag="sdrep")
            for h in range(2):
                prepd = pacc.tile([128, HALF], F32, tag="acc")
                nc.tensor.matmul(
                    prepd,
                    lhsT=ones,
                    rhs=s_d[:, ds(h * HALF, HALF)],
                    start=True,
                    stop=True,
                )
                nc.scalar.copy(sdrep[:, ds(h * HALF, HALF)], prepd)
            prec = psml.tile([128, BO], F32, tag="sml")
            nc.tensor.matmul(
                prec, lhsT=ones.bitcast(F32), rhs=rec_d, start=True, stop=True
            )
            rec_rep = one.tile([128, BO], F32, tag="rec_rep")
            nc.scalar.copy(rec_rep, prec)

            # weighted V-sum, normalize, and final linear - interleaved per c
            u_un = one.tile([128, C, BO], F32, tag="u_un")
            u_dT = one.tile([128, C, BO], F32R, tag="u_dT")
            OTf = OT.bitcast(F32).rearrange("p c b o l -> p c (b o) l")
            sdv = sdrep.rearrange("p (bo l) -> p bo l", l=LO)
            pout = psml.tile([BL, OUTP], F32, tag="sml")
            uv = u_dT.rearrange("p c (b o) -> p c b o", o=NOPT)
            for c in range(C):
                scrd = scratch.tile([128, BO, LO], F32, tag="scr")
                nc.vector.tensor_mul(scrd, OTf[:, c], sdv)
                nc.vector.tensor_reduce(
                    u_un[:, c : c + 1, :].rearrange("p one bo -> p bo one"),
                    scrd,
                    axis=AX.X,
                    op=ALU.add,
                )
                nc.vector.tensor_mul(u_dT[:, c, :], u_un[:, c, :], rec_rep)
                for o in range(NOPT):
                    nc.tensor.matmul(
                        pout,
                        lhsT=uv[:, c, :, o],
                        rhs=fw[:, o, c, :],
                        start=(c == 0 and o == 0),
                        stop=(c == C - 1 and o == NOPT - 1),
                    )
            out_s = one.tile([BL, OUTP], F32, tag="out_s")
            nc.vector.tensor_add(out_s, pout, fb)
            nc.sync.dma_start(out=outd, in_=out_s[:, 0:OUT])

    nc.compile()
    return nc


@functools.lru_cache(maxsize=1)
def get_nc() -> bass.Bass:
    return build_nc()


def make_in_maps(inputs: dict) -> list[dict]:
    art = np.ascontiguousarray(np.asarray(inputs["article_contexts"], np.float32))
    qc = np.ascontiguousarray(np.asarray(inputs["question_contexts"], np.float32))
    opt = np.ascontiguousarray(np.asarray(inputs["options_embeds"], np.float32))
    idx = np.asarray(inputs["answer_indices"]).astype(np.int64)

    def g(name):
        return np.asarray(inputs[name], np.float32)

    aQwT = np.ascontiguousarray(g("a_Qw").T)
    aKwT = np.ascontiguousarray(g("a_Kw").T)
    dKwT = np.ascontiguousarray(g("d_Kw").T)
    # folded: aq -> options query projection
    Wqv = g("d_Qw") @ g("a_Vw")  # [H, H] (a_Vw maps h_in->h_out as aq = u @ a_Vw^T)
    qvwT = np.ascontiguousarray(Wqv.T).astype(np.float32)
    bias_qv = g("d_Qw") @ g("a_Vb") + g("d_Qb") + g("d_Kb")  # [H]
    # folded: per-option final weights
    # feats[b,o,:] = u_d[b,o] @ d_Vw^T + d_Vb ; logits = sum_o feats[b,o] @ f_w[:,o]^T + f_b
    # => logits = sum_o u_d[b,o] @ (d_Vw^T @ f_w[:,o]^T) + (f_b + sum_o f_w[:,o] @ d_Vb)
    f_w = g("f_w")  # [OUT, 5H], flattened o-major
    dVwT = g("d_Vw").T  # [H_in, H_out]
    Ff = np.stack(
        [dVwT @ f_w[:, o * H : (o + 1) * H].T for o in range(NOPT)], axis=0
    )  # [o, H_in, OUT]
    fb_new = g("f_b") + sum(
        f_w[:, o * H : (o + 1) * H] @ g("d_Vb") for o in range(NOPT)
    )  # [OUT]
    fwT = np.zeros((128, NOPT, C, 8), np.float32)
    fwT[:, :, :, :OUT] = Ff.reshape(NOPT, C, 128, OUT).transpose(2, 0, 1, 3)

    def colvec(v):  # [H] -> [128, C] chunk-major
        return np.ascontiguousarray(np.asarray(v, np.float32).reshape(C, 128).T)

    vwaT = colvec(g("a_vw").reshape(H))
    vwdT = colvec(g("d_vw").reshape(H))
    qkbT = colvec(g("a_Qb") + g("a_Kb"))
    qvbT = colvec(bias_qv)

    artT = np.ascontiguousarray(art.transpose(0, 2, 1))  # [B, H, LA]
    optT = np.ascontiguousarray(opt.transpose(0, 3, 1, 2))  # [B, H, 5, LO]
    onehot = np.zeros((B, LQ), np.float32)
    onehot[np.arange(B), idx] = 1.0

    shared = dict(
        aQwT=aQwT, aKwT=aKwT, qvwT=qvwT, dKwT=dKwT,
        vwaT=vwaT, vwdT=vwdT, qkbT=qkbT, qvbT=qvbT,
        fwT=fwT,
        fb=np.ascontiguousarray(
            np.tile(
                np.pad(fb_new.astype(np.float32), (0, 3)).reshape(1, 8), (BL, 1)
            )
        ),
        ones1=np.ones((1, 128), np.float32),
    )
    in_maps = []
    for r in range(NCORES):
        s = slice(r * BL, (r + 1) * BL)
        m = dict(shared)
        m["artT"] = artT[s]
        m["optT"] = optT[s]
        m["qc"] = qc[s]
        m["oh"] = np.ascontiguousarray(onehot[s].T)
        in_maps.append(m)
    return in_maps


def run(inputs: dict, trace: bool = False, tmpdir=None):
    from concourse.bass_utils import run_bass_kernel_spmd

    nc = get_nc()
    in_maps = make_in_maps(inputs)
    res = run_bass_kernel_spmd(
        nc, in_maps, core_ids=list(range(NCORES)), trace=trace, tmpdir=tmpdir
    )
    out = np.concatenate([res.results[r]["out"] for r in range(NCORES)], axis=0)
    return out, res


def kernel(**inputs) -> np.ndarray:
    out, _ = run(inputs, trace=False)
    return out



# revision 33
# speedup vs baseline: 1.6830x; 1.6830x over previous
"""Trainium2 Bass kernel for nn_BaselineOut (article/option additive-attention MRC head).

Contract: kernel(**inputs) takes FULL unsharded inputs (numpy), returns FULL
[32, 5] float32 logits.  Internally: data-parallel over batch across 8 cores
(4 batch items per core), all params replicated.

Math notes (vs reference):
  - oqc gather is done as a one-hot matmul on device (host only encodes the
    int indices as a one-hot bf16 matrix - a layout/encoding transform).
  - V-projection is pulled out of the attention sum by linearity:
        sum_l softmax_l * (V @ Vw^T + Vb) = (sum_l softmax_l * V) @ Vw^T + Vb
    so the [B*L,H]x[H,H] V matmul collapses to a weighted sum over L plus a
    tiny [B,H]x[H,H] matmul.
  - Consecutive linear maps with no nonlinearity between are constant-folded
    on host (weight-weight products):
      * aq -> Qp_d: one matmul with Wqv = d_Qw @ a_Vw^T and a folded bias.
      * feats -> logits: per-option folded weights Ff_o = d_Vw^T @ f_w[:,o]^T.
  - softmax logit bias (vb) is dropped: softmax is shift-invariant.
  - exp is computed without max-subtraction: |logit| <= ||vw||_1 ~ 36, well
    inside fp32 exp range.
  - The big projections run in fp8e4 with DoubleRow perf mode (2 k-tiles per
    instruction, 0.5 cycles/row): article/option data and the K/Q projection
    weights are pre-cast to fp8 on host.  Weights are pre-scaled by 8 to
    stay in e4m3's normal range; the 1/8 (or 1/64) unscale folds into the
    activation-instruction `scale` operand for free.
  - The V-side weighted sums consume the SAME fp8 tiles (single HBM copy)
    via fused tensor_tensor_reduce on DVE, reading the PE-replicated
    softmax rows straight from PSUM.
  - The final linear keeps float32r operands: its inputs feed the output
    incoherently, so fp8 there would eat most of the 2e-2 error budget.
  - f32r must never be a DRAM I/O dtype (crashes NRT) - the f32->f32r casts
    (fw, ones) happen in SWDGE DMAs.
"""

import functools
import sys

import numpy as np

sys.path.insert(0, "/opt/trn_rl_repo")

import concourse.bass as bass  # noqa: E402
from concourse import bacc  # noqa: E402
import concourse.tile as tile  # noqa: E402
from concourse import mybir  # noqa: E402
from concourse.bass import ds, ts  # noqa: E402

B, LA, LQ, LO, H, OUT = 32, 2048, 64, 32, 1024, 5
NCORES = 8
BL = B // NCORES  # 4 batch items per core
NOPT = 5
F32 = mybir.dt.float32
F32R = mybir.dt.float32r
F8 = mybir.dt.float8e4
BF16 = mybir.dt.bfloat16
LT = 512  # article l-tile (free dim of the big matmuls; one PSUM bank)
NLT = LA // LT  # 4
NPAIR = NLT // 2  # 2 lt-pairs per batch item
C = H // 128  # 8 h-chunks
CP = C // 2  # 4 h-chunk pairs (DoubleRow)
BO = BL * NOPT  # 20 (b, option) pairs per core
AF = mybir.ActivationFunctionType
ALU = mybir.AluOpType
AX = mybir.AxisListType
DR = mybir.MatmulPerfMode.DoubleRow
OUTP = 8  # final-linear out dim padded even for f32r
WS = 8.0  # host-side fp8 weight pre-scale (power of two)
import os as _os  # noqa: E402

USE_TTR = _os.environ.get("KERN_NO_TTR", "") != "1"
USE_POOL = _os.environ.get("KERN_NO_POOL", "") != "1"


def build_nc() -> bass.Bass:
    nc = bacc.Bacc("TRN2", target_bir_lowering=False, debug=False)

    # ---- DRAM I/O (per-core shard; names are the in_map keys) ----
    # artT[b, p, t, c, l] = article[b, t*LT+l, c*128+p]: each (b, t) tile is
    # one DMA with a contiguous 4KB line per partition.
    artT = nc.dram_tensor("artT", [BL, 128, NLT, C, LT], F8, kind="ExternalInput").ap()
    optT = nc.dram_tensor(
        "optT", [128, C, BL, NOPT, LO], BF16, kind="ExternalInput"
    ).ap()
    qcd = nc.dram_tensor("qc", [LQ, BL, H], BF16, kind="ExternalInput").ap()
    ohd = nc.dram_tensor("oh", [LQ, BL], BF16, kind="ExternalInput").ap()
    wQa = nc.dram_tensor("aQwT", [128, C, H], F8, kind="ExternalInput").ap()
    wKa = nc.dram_tensor("aKwT", [128, C, H], F8, kind="ExternalInput").ap()
    wQV = nc.dram_tensor("qvwT", [128, C, H], F8, kind="ExternalInput").ap()
    wKd = nc.dram_tensor("dKwT", [128, C, H], BF16, kind="ExternalInput").ap()
    vwad = nc.dram_tensor("vwaT", [128, C], F8, kind="ExternalInput").ap()
    vwdd = nc.dram_tensor("vwdT", [128, C], F32, kind="ExternalInput").ap()
    qkbd = nc.dram_tensor("qkbT", [128, C], F32, kind="ExternalInput").ap()
    qvbd = nc.dram_tensor("qvbT", [128, C], F32, kind="ExternalInput").ap()
    fwd = nc.dram_tensor("fwT", [128, NOPT, C, OUTP], F32, kind="ExternalInput").ap()
    fbd = nc.dram_tensor("fb", [BL, OUTP], F32, kind="ExternalInput").ap()
    onesd = nc.dram_tensor("ones1", [1, 128], F32, kind="ExternalInput").ap()
    oneqd = nc.dram_tensor("oneq", [1, 128], F32, kind="ExternalInput").ap()
    outd = nc.dram_tensor("out", [BL, OUT], F32, kind="ExternalOutput").ap()

    with (
        tile.TileContext(nc) as tc,
        nc.allow_low_precision(reason="fp8 scores path; PE accumulates fp32"),
    ):
        with (
            tc.tile_pool(name="stream", bufs=6) as stream,
            tc.tile_pool(name="mtp", bufs=2) as mtp,
            tc.tile_pool(name="wbig", bufs=1) as wbig,
            tc.tile_pool(name="spool", bufs=2) as spool,
            tc.tile_pool(name="ubuf", bufs=2) as ubuf,
            tc.tile_pool(name="scratch", bufs=2) as scratch,
            tc.tile_pool(name="one", bufs=1) as one,
            tc.tile_pool(name="pacc", bufs=2, space="PSUM") as pacc,
            tc.tile_pool(name="prow", bufs=2, space="PSUM") as prow,
            tc.tile_pool(name="plg", bufs=2, space="PSUM") as plg,
        ):
            # ---------- small constant loads (ahead of big weights) ----------
            oht = one.tile([LQ, BL], BF16, tag="oht")
            nc.sync.dma_start(out=oht, in_=ohd)
            qct = one.tile([LQ, BL, H], BF16, tag="qct")
            nc.sync.dma_start(out=qct, in_=qcd)
            qkb = one.tile([128, C], F32, tag="qkb")
            nc.sync.dma_start(out=qkb, in_=qkbd)
            qvb = one.tile([128, C], F32, tag="qvb")
            nc.sync.dma_start(out=qvb, in_=qvbd)
            vwa = one.tile([128, C], F8, tag="vwa")
            nc.sync.dma_start(out=vwa, in_=vwad)
            vwd = one.tile([128, C], F32R, tag="vwd")
            nc.gpsimd.dma_start(out=vwd, in_=vwdd)
            oneq = one.tile([1, 128], F32, tag="oneq")
            nc.sync.dma_start(out=oneq, in_=oneqd)
            fb = one.tile([BL, OUTP], F32, tag="fb")
            nc.sync.dma_start(out=fb, in_=fbd)
            # f32 -> f32r casts ride SWDGE
            ones = one.tile([1, 128], F32R, tag="ones")
            nc.gpsimd.dma_start(out=ones, in_=onesd)
            fw = one.tile([128, NOPT, C, OUTP], F32R, tag="fw")
            nc.gpsimd.dma_start(out=fw, in_=fwd)

            # ---------- big weights (own DGE queue, parallel to sync) ----
            wq = wbig.tile([128, C, H], F8, tag="wq")
            nc.gpsimd.dma_start(out=wq, in_=wQa)
            wk = wbig.tile([128, C, H], F8, tag="wk")
            nc.gpsimd.dma_start(out=wk, in_=wKa)
            wqv = wbig.tile([128, C, H], F8, tag="wqv")
            wdk = wbig.tile([128, C, H], BF16, tag="wdk")

            # ---------- gather oqc via one-hot matmul ----------
            oqcT = one.tile([128, C, BL], F8, tag="oqcT")
            for c in range(C):
                po = pacc.tile([128, 2, LT], F32, tag="acc")
                for b in range(BL):
                    nc.tensor.matmul(
                        po[:, 0, 0:BL][:, b : b + 1],
                        lhsT=qct[:, b, ts(c, 128)],
                        rhs=oht[:, b : b + 1],
                        start=True,
                        stop=True,
                    )
                nc.vector.tensor_copy(oqcT[:, c, :], po[:, 0, 0:BL])

            # ---------- biasA = a_Qw @ oqc / WS + (a_Qb + a_Kb) ----------
            biasA = one.tile([128, C, BL], F32, tag="biasA")
            for co in range(C):
                pq = pacc.tile([128, 2, LT], F32, tag="acc")
                for j in range(CP):
                    nc.tensor.matmul(
                        pq[:, 0, 0:BL],
                        lhsT=wq[:, ds(2 * j, 2), ts(co, 128)],
                        rhs=oqcT[:, ds(2 * j, 2), :],
                        start=(j == 0),
                        stop=(j == CP - 1),
                        perf_mode=DR,
                    )
                nc.scalar.activation(
                    biasA[:, co, :],
                    pq[:, 0, 0:BL],
                    AF.Identity,
                    bias=qkb[:, co : co + 1],
                    scale=1.0 / WS,
                )

            # ---------- article branch ----------
            OT = one.tile([128, C, BL, NOPT, LO], BF16, tag="OT")
            s_sums = one.tile([1, BL, NLT], F32, tag="s_sums")
            uTun = one.tile([128, C, BL], F32, tag="uTun")
            for b in range(BL):
                upart = ubuf.tile([128, NLT, C], F32, tag="upart")
                for ltp in range(NPAIR):
                    Ts = []
                    for i in range(2):
                        T = stream.tile([128, C, LT], F8, tag="stream")
                        nc.sync.dma_start(out=T, in_=artT[b, :, 2 * ltp + i])
                        Ts.append(T)
                    if b == 0 and ltp == 0:
                        # big option-phase loads on the weights queue; they
                        # land mid-article, well before needed.
                        nc.gpsimd.dma_start(out=wqv, in_=wQV)
                        nc.gpsimd.dma_start(out=wdk, in_=wKd)
                        nc.gpsimd.dma_start(out=OT, in_=optT)
                    kp2 = pacc.tile([128, 2, LT], F32, tag="acc")
                    mt2 = mtp.tile([128, C, 2, LT], F8, tag="mt")
                    for co in range(C):
                        for i in range(2):
                            for j in range(CP):
                                nc.tensor.matmul(
                                    kp2[:, i, :],
                                    lhsT=wk[:, ds(2 * j, 2), ts(co, 128)],
                                    rhs=Ts[i][:, ds(2 * j, 2), :],
                                    start=(j == 0),
                                    stop=(j == CP - 1),
                                    perf_mode=DR,
                                )
                        # tanh over the full lt-pair in one wide instruction
                        nc.scalar.activation(
                            mt2[:, co],
                            kp2,
                            AF.Tanh,
                            bias=biasA[:, co, b : b + 1],
                            scale=1.0 / WS,
                        )
                    for i in range(2):
                        lt = 2 * ltp + i
                        lg = plg.tile([1, LT], F32, tag="lg")
                        for co in range(C):
                            nc.tensor.matmul(
                                lg,
                                lhsT=vwa[:, co : co + 1],
                                rhs=mt2[:, co, i, :],
                                start=(co == 0),
                                stop=(co == C - 1),
                            )
                        st = spool.tile([1, LT], F32R, tag="st")
                        nc.scalar.activation(
                            st,
                            lg,
                            AF.Exp,
                            scale=1.0 / WS,
                            accum_out=s_sums[:, b, lt : lt + 1],
                        )
                        # replicate s~ across partitions: ones^T (x) st via PE
                        prep = prow.tile([128, LT], F32, tag="prep")
                        nc.tensor.matmul(prep, lhsT=ones, rhs=st, start=True, stop=True)
                        # weighted V-sum: upart[:, lt, c] = sum_l T*s.
                        # (TensorTensorReduce crashes this runtime, so:
                        # products split DVE/Pool, then 2x-rate bf16 fold
                        # halvings and one short reduce on DVE.)
                        srep = spool.tile([128, LT], F32, tag="srep")
                        nc.vector.tensor_copy(srep, prep)
                        prod = scratch.tile([128, C, LT], BF16, tag="prod")
                        for c in range(3):
                            nc.vector.tensor_mul(prod[:, c], Ts[i][:, c, :], prep)
                        for c in range(3, C):
                            nc.gpsimd.tensor_mul(prod[:, c], Ts[i][:, c, :], srep)
                        fold1 = scratch.tile([128, C, LT // 2], BF16, tag="fold1")
                        nc.vector.tensor_add(
                            fold1, prod[:, :, 0 : LT // 2], prod[:, :, LT // 2 : LT]
                        )
                        fold2 = scratch.tile([128, C, LT // 4], BF16, tag="fold2")
                        nc.vector.tensor_add(
                            fold2,
                            fold1[:, :, 0 : LT // 4],
                            fold1[:, :, LT // 4 : LT // 2],
                        )
                        nc.vector.tensor_reduce(
                            upart[:, lt, :], fold2, axis=AX.X, op=ALU.add
                        )
                # sum the NLT partial weighted sums -> unnormalized u^T
                nc.vector.tensor_reduce(
                    uTun[:, :, b : b + 1],
                    upart.rearrange("p t c -> p c t"),
                    axis=AX.X,
                    op=ALU.add,
                )

            # ---------- options Kp matmuls (independent of the article
            # result - run during the article pipeline drain) ----------
            HALF = 2 * NOPT * LO  # 320 columns (2 batch items)
            mdt_pre = one.tile([128, C, BL, NOPT, LO], F32, tag="mdt_pre")
            for h in range(2):
                for cop in range(C // 2):
                    kpd = pacc.tile([128, 2, LT], F32, tag="acc")
                    for bank in range(2):
                        co = 2 * cop + bank
                        for ci in range(C):
                            nc.tensor.matmul(
                                kpd[:, bank, 0:HALF],
                                lhsT=wdk[:, ci, ts(co, 128)],
                                rhs=OT[:, ci, ds(2 * h, 2)],
                                start=(ci == 0),
                                stop=(ci == C - 1),
                            )
                        # park in SBUF until biasO is ready, splitting the
                        # copies across DVE and Act (both are draining here)
                        dst = mdt_pre[:, co, ds(2 * h, 2)].rearrange(
                            "p b o l -> p (b o l)"
                        )
                        if co % 2 == 0:
                            nc.vector.tensor_copy(dst, kpd[:, bank, 0:HALF])
                        else:
                            nc.scalar.copy(dst, kpd[:, bank, 0:HALF])

            # normalization: rs_rep = WS / sum(exp) per b, on all partitions
            ssb = one.tile([1, BL], F32, tag="ssb")
            nc.vector.tensor_reduce(ssb, s_sums, axis=AX.X, op=ALU.add)
            psb = prow.tile([128, LT], F32, tag="prep")
            nc.tensor.matmul(psb[:, 0:BL], lhsT=oneq, rhs=ssb, start=True, stop=True)
            rs_rep = one.tile([128, BL], F32, tag="rs_rep")
            nc.vector.reciprocal(rs_rep, psb[:, 0:BL])

            # uT8 = WS * u (normalized), fp8 for the biasO DoubleRow matmul
            uT = one.tile([128, C, BL], F8, tag="uT")
            for b in range(BL):
                nc.vector.tensor_scalar_mul(
                    uT[:, :, b], uTun[:, :, b], rs_rep[:, b : b + 1]
                )

            # ---------- option tanh bias via folded Wqv = d_Qw a_Vw^T ----------
            biasO = one.tile([128, C, BL], F32, tag="biasO")
            for co in range(C):
                pq2 = pacc.tile([128, 2, LT], F32, tag="acc")
                for j in range(CP):
                    nc.tensor.matmul(
                        pq2[:, 0, 0:BL],
                        lhsT=wqv[:, ds(2 * j, 2), ts(co, 128)],
                        rhs=uT[:, ds(2 * j, 2), :],
                        start=(j == 0),
                        stop=(j == CP - 1),
                        perf_mode=DR,
                    )
                nc.scalar.activation(
                    biasO[:, co, :],
                    pq2[:, 0, 0:BL],
                    AF.Identity,
                    bias=qvb[:, co : co + 1],
                    scale=1.0 / (WS * WS),
                )

            # ---------- options branch (bf16: fp8 here blows the 2e-2 gate) ----
            mdt = one.tile([128, C, BL, NOPT, LO], F32R, tag="mdt")
            for co in range(C):
                for b in range(BL):
                    nc.scalar.activation(
                        mdt[:, co, b],
                        mdt_pre[:, co, b],
                        AF.Tanh,
                        bias=biasO[:, co, b : b + 1],
                    )

            s_d = one.tile([1, BO * LO], F32R, tag="s_d")
            for h in range(2):
                lgd = plg.tile([1, LT], F32, tag="lg")
                for co in range(C):
                    nc.tensor.matmul(
                        lgd[:, 0:HALF],
                        lhsT=vwd[:, co : co + 1],
                        rhs=mdt[:, co, ds(2 * h, 2)],
                        start=(co == 0),
                        stop=(co == C - 1),
                    )
                nc.scalar.activation(
                    s_d[:, ds(h * HALF, HALF)], lgd[:, 0:HALF], AF.Exp
                )

            sums_d = one.tile([1, BO], F32, tag="sums_d")
            nc.vector.tensor_reduce(
                sums_d,
                s_d.bitcast(F32).rearrange("p (bo l) -> p bo l", l=LO),
                axis=AX.X,
                op=ALU.add,
            )
            rec_d = one.tile([1, BO], F32, tag="rec_d")
            nc.vector.reciprocal(rec_d, sums_d)
            # replicate raw exp scores and 1/sum across partitions
            sdrep = one.tile([128, BO * LO], F32, tag="sdrep")
            for h in range(2):
                prepd = prow.tile([128, LT], F32, tag="prep")
                nc.tensor.matmul(
                    prepd[:, 0:HALF],
                    lhsT=ones,
                    rhs=s_d[:, ds(h * HALF, HALF)],
                    start=True,
                    stop=True,
                )
                nc.scalar.copy(sdrep[:, ds(h * HALF, HALF)], prepd[:, 0:HALF])
            prec = prow.tile([128, LT], F32, tag="prep")
            nc.tensor.matmul(
                prec[:, 0:BO],
                lhsT=ones.bitcast(F32),
                rhs=rec_d,
                start=True,
                stop=True,
            )
            rec_rep = one.tile([128, BO], F32, tag="rec_rep")
            nc.scalar.copy(rec_rep, prec[:, 0:BO])

            # weighted V-sum, normalize, and final linear - interleaved per c
            u_un = one.tile([128, C, BO], F32, tag="u_un")
            u_dT = one.tile([128, C, BO], F32R, tag="u_dT")
            OTf = OT.rearrange("p c b o l -> p c (b o) l")
            sdv = sdrep.rearrange("p (bo l) -> p bo l", l=LO)
            poutt = pacc.tile([128, 2, LT], F32, tag="acc")
            pout = poutt[0:BL, 0, 0:OUTP]
            uv = u_dT.rearrange("p c (b o) -> p c b o", o=NOPT)
            for c in range(C):
                # Pool handles the product (it is idle here), DVE the reduce
                scrd = scratch.tile([128, BO, LO], F32, tag="scrd")
                (nc.gpsimd if USE_POOL else nc.vector).tensor_mul(
                    scrd, OTf[:, c], sdv
                )
                nc.vector.tensor_reduce(
                    u_un[:, c : c + 1, :].rearrange("p one bo -> p bo one"),
                    scrd,
                    axis=AX.X,
                    op=ALU.add,
                )
                nc.vector.tensor_mul(u_dT[:, c, :], u_un[:, c, :], rec_rep)
                for o in range(NOPT):
                    nc.tensor.matmul(
                        pout,
                        lhsT=uv[:, c, :, o],
                        rhs=fw[:, o, c, :],
                        start=(c == 0 and o == 0),
                        stop=(c == C - 1 and o == NOPT - 1),
                    )
            out_s = one.tile([BL, OUTP], F32, tag="out_s")
            nc.vector.tensor_add(out_s, pout, fb)
            nc.sync.dma_start(out=outd, in_=out_s[:, 0:OUT])

    nc.compile()
    return nc


@functools.lru_cache(maxsize=1)
def get_nc() -> bass.Bass:
    return build_nc()


F8NP = mybir.dt.np(F8)
BF16NP = mybir.dt.np(BF16)


def make_in_maps(inputs: dict) -> list[dict]:
    art = np.asarray(inputs["article_contexts"], np.float32)
    qc = np.asarray(inputs["question_contexts"], np.float32)
    opt = np.asarray(inputs["options_embeds"], np.float32)
    idx = np.asarray(inputs["answer_indices"]).astype(np.int64)

    def g(name):
        return np.asarray(inputs[name], np.float32)

    def wpack(w_T):  # [H_in, H_out] -> [128, C, H] fp8, pre-scaled by WS
        return np.ascontiguousarray(
            (w_T * WS).reshape(C, 128, H).transpose(1, 0, 2)
        ).astype(F8NP)

    def wpack_bf(w_T):  # [H_in, H_out] -> [128, C, H] bf16, unscaled
        return np.ascontiguousarray(
            w_T.reshape(C, 128, H).transpose(1, 0, 2)
        ).astype(BF16NP)

    aQwT = wpack(g("a_Qw").T)
    aKwT = wpack(g("a_Kw").T)
    dKwT = wpack_bf(g("d_Kw").T)
    # folded: aq -> options query projection
    Wqv = g("d_Qw") @ g("a_Vw")  # [H, H] (a_Vw maps h_in->h_out as aq = u @ a_Vw^T)
    qvwT = wpack(Wqv.T)
    bias_qv = g("d_Qw") @ g("a_Vb") + g("d_Qb") + g("d_Kb")  # [H]
    # folded: per-option final weights
    # feats[b,o,:] = u_d[b,o] @ d_Vw^T + d_Vb ; logits = sum_o feats[b,o] @ f_w[:,o]^T + f_b
    # => logits = sum_o u_d[b,o] @ (d_Vw^T @ f_w[:,o]^T) + (f_b + sum_o f_w[:,o] @ d_Vb)
    f_w = g("f_w")  # [OUT, 5H], flattened o-major
    dVwT = g("d_Vw").T  # [H_in, H_out]
    Ff = np.stack(
        [dVwT @ f_w[:, o * H : (o + 1) * H].T for o in range(NOPT)], axis=0
    )  # [o, H_in, OUT]
    fb_new = g("f_b") + sum(
        f_w[:, o * H : (o + 1) * H] @ g("d_Vb") for o in range(NOPT)
    )  # [OUT]
    fwT = np.zeros((128, NOPT, C, 8), np.float32)
    fwT[:, :, :, :OUT] = Ff.reshape(NOPT, C, 128, OUT).transpose(2, 0, 1, 3)

    def colvec(v, dtype=np.float32, scale=1.0):  # [H] -> [128, C] chunk-major
        return np.ascontiguousarray(
            (np.asarray(v, np.float32) * scale).reshape(C, 128).T
        ).astype(dtype)

    vwaT = colvec(g("a_vw").reshape(H), F8NP, WS)
    vwdT = colvec(g("d_vw").reshape(H))
    qkbT = colvec(g("a_Qb") + g("a_Kb"))
    qvbT = colvec(bias_qv)

    # artT[b, p, t, c, l] = art[b, t*LT+l, c*128+p]
    artT = np.ascontiguousarray(
        art.transpose(0, 2, 1)  # [B, H, LA]
        .reshape(B, C, 128, NLT, LT)
        .transpose(0, 2, 3, 1, 4)
    ).astype(F8NP)
    # optT[b][p, c, bb, o, l] built per-core below from [B, 128, C, 5, LO]
    optP = (
        opt.transpose(0, 3, 1, 2)  # [B, H, 5, LO]
        .reshape(B, C, 128, NOPT, LO)
        .transpose(0, 2, 1, 3, 4)  # [B, 128, C, 5, LO]
    ).astype(BF16NP)
    qcP = np.ascontiguousarray(qc.transpose(1, 0, 2)).astype(BF16NP)  # [LQ, B, H]
    onehot = np.zeros((LQ, B), np.float32)
    onehot[idx, np.arange(B)] = 1.0
    onehot = onehot.astype(BF16NP)

    shared = dict(
        aQwT=aQwT, aKwT=aKwT, qvwT=qvwT, dKwT=dKwT,
        vwaT=vwaT, vwdT=vwdT, qkbT=qkbT, qvbT=qvbT,
        fwT=fwT,
        fb=np.ascontiguousarray(
            np.tile(
                np.pad(fb_new.astype(np.float32), (0, 3)).reshape(1, 8), (BL, 1)
            )
        ),
        ones1=np.ones((1, 128), np.float32),
        oneq=np.full((1, 128), 1.0 / WS, np.float32),
    )
    in_maps = []
    for r in range(NCORES):
        s = slice(r * BL, (r + 1) * BL)
        m = dict(shared)
        m["artT"] = artT[s]
        m["optT"] = np.ascontiguousarray(
            optP[s].transpose(1, 2, 0, 3, 4)
        )  # [128, C, BL, 5, LO]
        m["qc"] = np.ascontiguousarray(qcP[:, s])
        m["oh"] = np.ascontiguousarray(onehot[:, s])
        in_maps.append(m)
    return in_maps


def run(inputs: dict, trace: bool = False, tmpdir=None):
    from concourse.bass_utils import run_bass_kernel_spmd

    nc = get_nc()
    in_maps = make_in_maps(inputs)
    res = run_bass_kernel_spmd(
        nc, in_maps, core_ids=list(range(NCORES)), trace=trace, tmpdir=tmpdir
    )
    out = np.concatenate([res.results[r]["out"] for r in range(NCORES)], axis=0)
    return out, res


def kernel(**inputs) -> np.ndarray:
    out, _ = run(inputs, trace=False)
    return out
